# revision 11
# baseline (speedup 1.0000x reference)
"""Trainium2 Bass kernel for nn_CRA_46797963657479.

Math: the reference builds per-batch gram matrix A = cat_phi^T cat_phi and
feeds concat(A, A^T) through big 1x1 convs.  A is symmetric and the tail is
linear, so it collapses to

    W[b, l] = (u3 + cat_phi[b] @ u4) . cat_phi[b][:, l] + K
    out[b]  = xp[b] * W[b, :N] + yp[b] * W[b, N:]

with u3 = w5a @ w3, u4 = w5b @ (w4[:, :2N] + w4[:, 2N:]),
K = w5a.b3 + w5b.b4 + b5; BN folds into the conv weights.

v2 design notes (per pair of batches A,B):
  PE     8 phi matmuls (bias folded via a ones-row in the hi contraction
         chunk: HBM x carries a constant-1.0 row 192) + 4 W matmuls.
  Scalar 2 relu activations PSUM->SBUF bf16 (784 cols each) + W copy
         PSUM->SBUF bf16 (scalar is the fast PSUM reader; GpSimd has no
         PSUM port at all).
  DVE    4 scalar_tensor_tensor z-reductions (junk out, accum per
         batch-chunk), one tiny q=z+u3 add, and the two combine
         multiplies t = x*W (all-bf16 SBUF -> 2x mode).
  GpSimd the final stream add og = t1 + t2 (SBUF only).
  Sync   all DMA issues.

All I/O is bf16 (inputs converted host-side, output upcast host-side);
HBM layouts are channel-major so every DMA row is contiguous.
Sharding: pure data parallel, batch 256 -> 32 per core on 8 cores.
"""

import ml_dtypes
import numpy as np

import concourse.bass as bass
import concourse.bacc as bacc
import concourse.tile as tile
from concourse import mybir
from concourse.bass_utils import run_bass_kernel_spmd

F32 = mybir.dt.float32
BF16 = mybir.dt.bfloat16

B, N, C = 256, 196, 192
NCORES = 8
NB = B // NCORES          # 32 batches per core
NPAIR = NB // 2           # 16 pairs per core
L = 2 * N                 # 392
CLO, CHI = 128, C - 128   # 128 + 64 channel split
CHIA = CHI + 1            # hi chunk + ones row
GP = 4                    # pairs per I/O group (8 batches)
NGRP = NPAIR // GP

_CACHE = {}


def _reap(ap, dims):
    """Rebuild an AP with explicit free-dim [stride, n] list."""
    return bass.AP(tensor=ap.tensor, offset=ap.offset, ap=[ap.ap[0]] + dims)


def _build_program():
    nc = bacc.Bacc("TRN2", target_bir_lowering=False, debug=False)

    # HBM layouts are channel-major; xyb row 192 is constant 1.0 (the
    # ones-row that folds the conv bias into the hi-contraction matmul).
    xyb = nc.dram_tensor("xyb", [C + 1, NB, 2, N], BF16, kind="ExternalInput")
    outt = nc.dram_tensor("out", [C, NB, N], BF16, kind="ExternalOutput")
    wblob = nc.dram_tensor("wblob", [CLO, 772], BF16, kind="ExternalInput")
    u4bT = nc.dram_tensor("u4b", [CLO, L], BF16, kind="ExternalInput")
    u34T = nc.dram_tensor("u34", [CLO, 4], F32, kind="ExternalInput")

    relu = mybir.ActivationFunctionType.Relu
    cpy = mybir.ActivationFunctionType.Copy
    mult = mybir.AluOpType.mult
    add = mybir.AluOpType.add
    byp = mybir.AluOpType.bypass

    with tile.TileContext(nc) as tc:
        with (
            tc.tile_pool(name="consts", bufs=1) as consts,
            tc.tile_pool(name="xin", bufs=2) as xin,
            tc.tile_pool(name="phip", bufs=3) as phip,
            tc.tile_pool(name="junkp", bufs=2) as junkp,
            tc.tile_pool(name="qp", bufs=4) as qp,
            tc.tile_pool(name="wsbp", bufs=3) as wsbp,
            tc.tile_pool(name="work", bufs=3) as work,
            tc.tile_pool(name="outp", bufs=2) as outp,
            tc.tile_pool(name="psph", bufs=1, space="PSUM") as psph,
            tc.tile_pool(name="psw", bufs=1, space="PSUM") as psw,
        ):
            wb = consts.tile([CLO, 772], BF16)
            nc.sync.dma_start(out=wb[:], in_=wblob[:])
            u4t = consts.tile([CLO, L], BF16)
            nc.sync.dma_start(out=u4t[:], in_=u4bT[:])
            u3t = consts.tile([CLO, 4], F32)
            nc.sync.dma_start(out=u3t[:], in_=u34T[:])

            twxA = wb[:, 0:128]
            twxB = wb[:, 128:193]
            twxC = wb[0:CHIA, 193:321]
            twxD = wb[0:CHIA, 321:386]
            twyA = wb[:, 386:514]
            twyB = wb[:, 514:579]
            twyC = wb[0:CHIA, 579:707]
            twyD = wb[0:CHIA, 707:772]
            # u4 flat [392]
            u4f_lo = _reap(u4t[:], [[1, L]])
            u4f_hi = _reap(u4t[0:CHIA, :], [[1, L]])

            # PE HAM warm-up: dense matmuls so the clock promotes to 2.4GHz
            # before the first real matmul.
            wseed = consts.tile([CLO, 640], BF16)
            nc.vector.memset(wseed[:], 1.0)
            wup = psw.tile([CLO, 2, 512], F32, tag="ps_w")
            for _ in range(24):
                nc.tensor.matmul(wup[:, 0, :], wseed[:, 512:640],
                                 wseed[:, 0:512], start=True, stop=True)

            def emit_front(p, xgb, og):
                u = p % GP
                b0 = 2 * u
                x_lo = xgb[:, 0, b0:b0 + 2, 0, :]
                x_hi = xgb[0:CHIA, 1, b0:b0 + 2, 0, :]
                y_lo = xgb[:, 0, b0:b0 + 2, 1, :]
                y_hi = xgb[0:CHIA, 1, b0:b0 + 2, 1, :]

                ps_lo = psph.tile([CLO, 2, 512], F32, tag=f"ps_lo{p % 2}")
                ps_hi = psph.tile([CHIA, 2, 512], F32, tag="ps_hi")
                nc.tensor.matmul(ps_lo[:, 0, 0:L], twxA, x_lo, start=True, stop=False)
                nc.tensor.matmul(ps_lo[:, 0, 0:L], twxC, x_hi, start=False, stop=True)
                nc.tensor.matmul(ps_lo[:, 1, 0:L], twyA, y_lo, start=True, stop=False)
                nc.tensor.matmul(ps_lo[:, 1, 0:L], twyC, y_hi, start=False, stop=True)
                nc.tensor.matmul(ps_hi[:, 0, 0:L], twxB, x_lo, start=True, stop=False)
                nc.tensor.matmul(ps_hi[:, 0, 0:L], twxD, x_hi, start=False, stop=True)
                nc.tensor.matmul(ps_hi[:, 1, 0:L], twyB, y_lo, start=True, stop=False)
                nc.tensor.matmul(ps_hi[:, 1, 0:L], twyD, y_hi, start=False, stop=True)

                # phi [p, chunk, b, s, n]; one act per chunk, relu, bf16 out.
                phi = phip.tile([CLO, 2, 2, 2, N], BF16, tag="phi")
                in_lo = _reap(ps_lo[:, :, 0:L], [[196, 2], [512, 2], [1, 196]])
                in_hi = _reap(ps_hi[:, :, 0:L], [[196, 2], [512, 2], [1, 196]])
                nc.scalar.activation(phi[:, 0, :, :, :], in_lo, relu)
                nc.scalar.activation(phi[0:CHIA, 1, :, :, :], in_hi, relu)

                # z[c] = sum_l phi[c,l]*u4[l] per batch-chunk: one flat stt
                # (junk out + per-partition accum) per batch-chunk.
                junk = junkp.tile([CLO, 2, 2, 2, N], BF16, tag="junk")
                z4 = qp.tile([CLO, 4], F32, tag="z4")
                if p < 4:
                    nc.vector.memset(z4[:], 0.0)
                for b in (0, 1):
                    nc.vector.scalar_tensor_tensor(
                        out=_reap(junk[:, 0, b, :, :], [[1, L]]),
                        in0=_reap(phi[:, 0, b, :, :], [[1, L]]),
                        scalar=1.0, in1=u4f_lo, op0=byp, op1=mult,
                        accum_out=z4[:, b:b + 1])
                    nc.vector.scalar_tensor_tensor(
                        out=_reap(junk[0:CHIA, 1, b, :, :], [[1, L]]),
                        in0=_reap(phi[0:CHIA, 1, b, :, :], [[1, L]]),
                        scalar=1.0, in1=u4f_hi, op0=byp, op1=mult,
                        accum_out=z4[0:CHIA, 2 + b:3 + b])
                qb = qp.tile([CLO, 4], BF16, tag="qb")
                nc.gpsimd.tensor_tensor(qb[:], z4[:], u3t[:], add)
                return dict(p=p, xgb=xgb, og=og, phi=phi, qb=qb)

            def emit_back(st, wsbq):
                p, xgb, og, phi, qb = st["p"], st["xgb"], st["og"], st["phi"], st["qb"]
                u = p % GP
                v = u % 2          # pair within quad
                ps_w = psw.tile([CLO, 2, 512], F32, tag="ps_w")
                for b in (0, 1):
                    nc.tensor.matmul(ps_w[:, b, 0:L],
                                     qb[:, b:b + 1].broadcast_to([CLO, CLO]),
                                     phi[:, 0, b, :, :], start=True, stop=False)
                    nc.tensor.matmul(ps_w[:, b, 0:L],
                                     qb[0:CHIA, 2 + b:3 + b].broadcast_to([CHIA, CLO]),
                                     phi[0:CHIA, 1, b, :, :], start=False, stop=True)

                # W -> SBUF bf16 (scalar engine: fast PSUM reader); two pairs
                # fill one quad-wide wsb tile.
                nc.scalar.activation(wsbq[:, 2 * v:2 * v + 2, :], ps_w[:, :, 0:L], cpy)

                # HAM keep-warm fillers: dense dummy matmuls into the retired
                # ps_hi generation so the PE never idles long enough for the
                # clock gate to drop back to 1.2GHz.
                fill = psph.tile([CHIA, 2, 512], F32, tag="ps_hi")
                for s in (0, 1):
                    nc.tensor.matmul(fill[:, s, :], wseed[:, 0:CHIA],
                                     wseed[:, 0:512], start=True, stop=True)

                if v == 1:
                    # combine at quad granularity: og = x*Wx + y*Wy
                    b0 = 4 * ((u // 2) % 2)
                    w_x = _reap(wsbq[:], [[0, 2], [L, 4], [1, 196]])
                    w_y = _reap(wsbq[:, :, 196:L], [[0, 2], [L, 4], [1, 196]])
                    t1 = work.tile([CLO, 2, 4, N], BF16, tag="t1")
                    t2 = work.tile([CLO, 2, 4, N], BF16, tag="t2")
                    nc.vector.tensor_tensor(t1[:], xgb[:, :, b0:b0 + 4, 0, :], w_x, mult)
                    nc.vector.tensor_tensor(t2[:], xgb[:, :, b0:b0 + 4, 1, :], w_y, mult)
                    nc.gpsimd.tensor_tensor(og[:, :, b0:b0 + 4, :], t1[:], t2[:], add)
                if u == GP - 1:
                    gb = 2 * GP * (p // GP)
                    nc.sync.dma_start(out=outt[0:CLO, gb:gb + 2 * GP, :],
                                      in_=og[:, 0, :, :])
                    nc.sync.dma_start(out=outt[CLO:C, gb:gb + 2 * GP, :],
                                      in_=og[0:CHI, 1, :, :])

            LAG = 2
            pending = []
            wsb_cur = [None]

            def pop_back(st):
                if st["p"] % 2 == 0:
                    wsb_cur[0] = wsbp.tile([CLO, 4, L], BF16, tag="wsb", name="wsbq")
                emit_back(st, wsb_cur[0])

            for g in range(NGRP):
                gb = 2 * GP * g
                xgb = xin.tile([CLO, 2, 2 * GP, 2, N], BF16, tag="xgb")
                nc.sync.dma_start(out=xgb[:, 0, :, :, :],
                                  in_=xyb[0:CLO, gb:gb + 2 * GP, :, :])
                nc.sync.dma_start(out=xgb[0:CHIA, 1, :, :, :],
                                  in_=xyb[CLO:C + 1, gb:gb + 2 * GP, :, :])
                og = outp.tile([CLO, 2, 2 * GP, N], BF16, tag="og")
                for u in range(GP):
                    pending.append(emit_front(GP * g + u, xgb, og))
                    if len(pending) > LAG:
                        pop_back(pending.pop(0))
            for st in pending:
                pop_back(st)

    nc.compile()
    return nc


def _host_prepack(d):
    """Fold BN, collapse the linear tail, build constant blobs."""
    f = np.float32
    bf = ml_dtypes.bfloat16
    inv1 = d["g1"] / np.sqrt(d["v1"] + 1e-5)
    W1 = (d["w1"] * inv1[:, None]).astype(f)
    c1 = ((d["b1"] - d["m1"]) * inv1 + d["be1"]).astype(f)
    inv2 = d["g2"] / np.sqrt(d["v2"] + 1e-5)
    W2 = (d["w2"] * inv2[:, None]).astype(f)
    c2 = ((d["b2"] - d["m2"]) * inv2 + d["be2"]).astype(f)

    w4eff = d["w4"][:, :L] + d["w4"][:, L:]
    w5a, w5b = d["w5"][0, :C], d["w5"][0, C:]
    u3 = (w5a @ d["w3"]).astype(f)
    u4 = (w5b @ w4eff).astype(f)
    K = float(w5a @ d["b3"] + w5b @ d["b4"] + d["b5"][0])

    W1T = np.ascontiguousarray(W1.T)
    W2T = np.ascontiguousarray(W2.T)

    wblob = np.zeros((CLO, 772), bf)

    def pack_stream(c0, WT, cb):
        # A [128,128]
        wblob[:, c0:c0 + 128] = WT[0:128, 0:128].astype(bf)
        # B [128,65] (out-hi; ones-unit comes via row64 of D)
        wblob[:, c0 + 128:c0 + 192] = WT[0:128, 128:192].astype(bf)
        # C [65,128]: rows 0:64 = WT hi-contraction, row 64 = bias
        wblob[0:CHI, c0 + 193:c0 + 321] = WT[128:192, 0:128].astype(bf)
        wblob[CHI, c0 + 193:c0 + 321] = cb[0:128].astype(bf)
        # D [65,65]
        wblob[0:CHI, c0 + 321:c0 + 385] = WT[128:192, 128:192].astype(bf)
        wblob[CHI, c0 + 321:c0 + 385] = cb[128:192].astype(bf)
        wblob[CHI, c0 + 385] = bf(1.0)

    pack_stream(0, W1T, c1)
    pack_stream(386, W2T, c2)

    u4b = np.broadcast_to(u4.astype(bf), (CLO, L)).copy()
    u34 = np.zeros((CLO, 4), f)
    u34[:, 0] = u34[:, 1] = u3[0:128]
    u34[0:CHI, 2] = u34[0:CHI, 3] = u3[128:192]
    u34[CHI, 2] = u34[CHI, 3] = K - float(u4.sum())
    return {"wblob": wblob, "u4b": u4b, "u34": u34}


def run(inputs, trace=False):
    d = {k: np.asarray(v) for k, v in inputs.items()}
    consts = _host_prepack(d)
    bf = ml_dtypes.bfloat16

    xyb = np.ones((C + 1, B, 2, N), bf)
    xyb[0:C, :, 0, :] = d["x"].transpose(2, 0, 1).astype(bf)
    xyb[0:C, :, 1, :] = d["y"].transpose(2, 0, 1).astype(bf)

    if "nc" not in _CACHE:
        _CACHE["nc"] = _build_program()
    nc = _CACHE["nc"]

    in_maps = []
    for cid in range(NCORES):
        m = dict(consts)
        m["xyb"] = np.ascontiguousarray(xyb[:, cid * NB:(cid + 1) * NB])
        in_maps.append(m)

    res = run_bass_kernel_spmd(nc, in_maps, list(range(NCORES)), trace=trace)
    # out HBM is [C, NB, N] bf16 channel-major; reassemble [B, C, N] f32
    out = np.concatenate(
        [res.results[i]["out"].astype(np.float32) for i in range(NCORES)], axis=1
    ).transpose(1, 0, 2)
    return np.ascontiguousarray(out), res


def kernel(**inputs):
    out, _ = run(inputs, trace=False)
    return out


# revision 12
# speedup vs baseline: 1.0527x; 1.0527x over previous
"""Trainium2 Bass kernel for nn_CRA_46797963657479.

Math: the reference builds per-batch gram matrix A = cat_phi^T cat_phi and
feeds concat(A, A^T) through big 1x1 convs.  A is symmetric and the tail is
linear, so it collapses to

    W[b, l] = (u3 + cat_phi[b] @ u4) . cat_phi[b][:, l] + K
    out[b]  = xp[b] * W[b, :N] + yp[b] * W[b, N:]

with u3 = w5a @ w3, u4 = w5b @ (w4[:, :2N] + w4[:, 2N:]),
K = w5a.b3 + w5b.b4 + b5; BN folds into the conv weights.

v2 design notes (per pair of batches A,B):
  PE     8 phi matmuls (bias folded via a ones-row in the hi contraction
         chunk: HBM x carries a constant-1.0 row 192) + 4 W matmuls.
  Scalar 2 relu activations PSUM->SBUF bf16 (784 cols each) + W copy
         PSUM->SBUF bf16 (scalar is the fast PSUM reader; GpSimd has no
         PSUM port at all).
  DVE    4 scalar_tensor_tensor z-reductions (junk out, accum per
         batch-chunk), one tiny q=z+u3 add, and the two combine
         multiplies t = x*W (all-bf16 SBUF -> 2x mode).
  GpSimd the final stream add og = t1 + t2 (SBUF only).
  Sync   all DMA issues.

All I/O is bf16 (inputs converted host-side, output upcast host-side);
HBM layouts are channel-major so every DMA row is contiguous.
Sharding: pure data parallel, batch 256 -> 32 per core on 8 cores.
"""

import ml_dtypes
import numpy as np

import concourse.bass as bass
import concourse.bacc as bacc
import concourse.tile as tile
from concourse import mybir
from concourse.bass_utils import run_bass_kernel_spmd

F32 = mybir.dt.float32
BF16 = mybir.dt.bfloat16

B, N, C = 256, 196, 192
NCORES = 8
NB = B // NCORES          # 32 batches per core
NPAIR = NB // 2           # 16 pairs per core
L = 2 * N                 # 392
CLO, CHI = 128, C - 128   # 128 + 64 channel split
CHIA = CHI + 1            # hi chunk + ones row
GP = 4                    # pairs per I/O group (8 batches)
NGRP = NPAIR // GP

_CACHE = {}


def _reap(ap, dims):
    """Rebuild an AP with explicit free-dim [stride, n] list."""
    return bass.AP(tensor=ap.tensor, offset=ap.offset, ap=[ap.ap[0]] + dims)


def _build_program():
    nc = bacc.Bacc("TRN2", target_bir_lowering=False, debug=False)

    # HBM layouts are channel-major; xyb row 192 is constant 1.0 (the
    # ones-row that folds the conv bias into the hi-contraction matmul).
    xyb = nc.dram_tensor("xyb", [C + 1, NB, 2, N], BF16, kind="ExternalInput")
    outt = nc.dram_tensor("out", [C, NB, N], BF16, kind="ExternalOutput")
    wblob = nc.dram_tensor("wblob", [CLO, 772], BF16, kind="ExternalInput")
    u4bT = nc.dram_tensor("u4b", [CLO, L], BF16, kind="ExternalInput")
    u34T = nc.dram_tensor("u34", [CLO, 4], F32, kind="ExternalInput")

    relu = mybir.ActivationFunctionType.Relu
    cpy = mybir.ActivationFunctionType.Copy
    mult = mybir.AluOpType.mult
    add = mybir.AluOpType.add
    byp = mybir.AluOpType.bypass

    with tile.TileContext(nc) as tc:
        with (
            tc.tile_pool(name="consts", bufs=1) as consts,
            tc.tile_pool(name="xin", bufs=2) as xin,
            tc.tile_pool(name="phip", bufs=3) as phip,
            tc.tile_pool(name="junkp", bufs=2) as junkp,
            tc.tile_pool(name="qp", bufs=4) as qp,
            tc.tile_pool(name="wsbp", bufs=3) as wsbp,
            tc.tile_pool(name="work", bufs=3) as work,
            tc.tile_pool(name="outp", bufs=2) as outp,
            tc.tile_pool(name="psph", bufs=1, space="PSUM") as psph,
            tc.tile_pool(name="psw", bufs=1, space="PSUM") as psw,
        ):
            wb = consts.tile([CLO, 772], BF16)
            nc.sync.dma_start(out=wb[:], in_=wblob[:])
            u4t = consts.tile([CLO, L], BF16)
            nc.sync.dma_start(out=u4t[:], in_=u4bT[:])
            u3t = consts.tile([CLO, 4], F32)
            nc.sync.dma_start(out=u3t[:], in_=u34T[:])

            twxA = wb[:, 0:128]
            twxB = wb[:, 128:193]
            twxC = wb[0:CHIA, 193:321]
            twxD = wb[0:CHIA, 321:386]
            twyA = wb[:, 386:514]
            twyB = wb[:, 514:579]
            twyC = wb[0:CHIA, 579:707]
            twyD = wb[0:CHIA, 707:772]
            # u4 flat [392]
            u4f_lo = _reap(u4t[:], [[1, L]])
            u4f_hi = _reap(u4t[0:CHIA, :], [[1, L]])

            # PE HAM warm-up: dense matmuls so the clock promotes to 2.4GHz
            # before the first real matmul.
            wseed = consts.tile([CLO, 640], BF16)
            nc.vector.memset(wseed[:], 1.0)
            wup = psw.tile([CLO, 2, 512], F32, tag="ps_w")
            for _ in range(24):
                nc.tensor.matmul(wup[:, 0, :], wseed[:, 512:640],
                                 wseed[:, 0:512], start=True, stop=True)

            def emit_front(p, xgb, og):
                u = p % GP
                b0 = 2 * u
                x_lo = xgb[:, 0, b0:b0 + 2, 0, :]
                x_hi = xgb[0:CHIA, 1, b0:b0 + 2, 0, :]
                y_lo = xgb[:, 0, b0:b0 + 2, 1, :]
                y_hi = xgb[0:CHIA, 1, b0:b0 + 2, 1, :]

                ps_lo = psph.tile([CLO, 2, 512], F32, tag=f"ps_lo{p % 2}")
                ps_hi = psph.tile([CHIA, 2, 512], F32, tag="ps_hi")
                nc.tensor.matmul(ps_lo[:, 0, 0:L], twxA, x_lo, start=True, stop=False)
                nc.tensor.matmul(ps_lo[:, 0, 0:L], twxC, x_hi, start=False, stop=True)
                nc.tensor.matmul(ps_lo[:, 1, 0:L], twyA, y_lo, start=True, stop=False)
                nc.tensor.matmul(ps_lo[:, 1, 0:L], twyC, y_hi, start=False, stop=True)
                nc.tensor.matmul(ps_hi[:, 0, 0:L], twxB, x_lo, start=True, stop=False)
                nc.tensor.matmul(ps_hi[:, 0, 0:L], twxD, x_hi, start=False, stop=True)
                nc.tensor.matmul(ps_hi[:, 1, 0:L], twyB, y_lo, start=True, stop=False)
                nc.tensor.matmul(ps_hi[:, 1, 0:L], twyD, y_hi, start=False, stop=True)

                # phi [p, chunk, b, s, n]; one act per chunk, relu, bf16 out.
                phi = phip.tile([CLO, 2, 2, 2, N], BF16, tag="phi")
                in_lo = _reap(ps_lo[:, :, 0:L], [[196, 2], [512, 2], [1, 196]])
                in_hi = _reap(ps_hi[:, :, 0:L], [[196, 2], [512, 2], [1, 196]])
                nc.scalar.activation(phi[:, 0, :, :, :], in_lo, relu)
                nc.scalar.activation(phi[0:CHIA, 1, :, :, :], in_hi, relu)

                # z[c] = sum_l phi[c,l]*u4[l] per batch-chunk: one flat stt
                # (junk out + per-partition accum) per batch-chunk.
                junk = junkp.tile([CLO, 2, 2, 2, N], BF16, tag="junk")
                z4 = qp.tile([CLO, 4], F32, tag="z4")
                if p < 4:
                    nc.vector.memset(z4[:], 0.0)
                for b in (0, 1):
                    nc.vector.scalar_tensor_tensor(
                        out=_reap(junk[:, 0, b, :, :], [[1, L]]),
                        in0=_reap(phi[:, 0, b, :, :], [[1, L]]),
                        scalar=1.0, in1=u4f_lo, op0=byp, op1=mult,
                        accum_out=z4[:, b:b + 1])
                    nc.vector.scalar_tensor_tensor(
                        out=_reap(junk[0:CHIA, 1, b, :, :], [[1, L]]),
                        in0=_reap(phi[0:CHIA, 1, b, :, :], [[1, L]]),
                        scalar=1.0, in1=u4f_hi, op0=byp, op1=mult,
                        accum_out=z4[0:CHIA, 2 + b:3 + b])
                qb = qp.tile([CLO, 4], BF16, tag="qb")
                nc.gpsimd.tensor_tensor(qb[:], z4[:], u3t[:], add)
                return dict(p=p, xgb=xgb, og=og, phi=phi, qb=qb)

            def emit_back(st):
                p, xgb, og, phi, qb = st["p"], st["xgb"], st["og"], st["phi"], st["qb"]
                u = p % GP
                b0 = 2 * u
                ps_w = psw.tile([CLO, 2, 512], F32, tag="ps_w")
                for b in (0, 1):
                    nc.tensor.matmul(ps_w[:, b, 0:L],
                                     qb[:, b:b + 1].broadcast_to([CLO, CLO]),
                                     phi[:, 0, b, :, :], start=True, stop=False)
                    nc.tensor.matmul(ps_w[:, b, 0:L],
                                     qb[0:CHIA, 2 + b:3 + b].broadcast_to([CHIA, CLO]),
                                     phi[0:CHIA, 1, b, :, :], start=False, stop=True)

                # W -> SBUF bf16 (scalar engine: fast PSUM reader)
                wsb = wsbp.tile([CLO, 2, L], BF16, tag="wsb")
                nc.scalar.activation(wsb[:], ps_w[:, :, 0:L], cpy)

                # combine: og = x*Wx + y*Wy over [chunk, b, n]
                w_x = _reap(wsb[:], [[0, 2], [L, 2], [1, 196]])
                w_y = _reap(wsb[:, :, 196:L], [[0, 2], [L, 2], [1, 196]])
                t1 = work.tile([CLO, 2, 2, N], BF16, tag="t1")
                t2 = work.tile([CLO, 2, 2, N], BF16, tag="t2")
                nc.vector.tensor_tensor(t1[:], xgb[:, :, b0:b0 + 2, 0, :], w_x, mult)
                nc.vector.tensor_tensor(t2[:], xgb[:, :, b0:b0 + 2, 1, :], w_y, mult)
                nc.gpsimd.tensor_tensor(og[:, :, b0:b0 + 2, :], t1[:], t2[:], add)
                # HAM keep-warm fillers
                fill = psph.tile([CHIA, 2, 512], F32, tag="ps_hi")
                for s in (0, 1):
                    nc.tensor.matmul(fill[:, s, :], wseed[:, 0:CHIA],
                                     wseed[:, 0:512], start=True, stop=True)
                if u == GP - 1:
                    gb = 2 * GP * (p // GP)
                    nc.sync.dma_start(out=outt[0:CLO, gb:gb + 2 * GP, :],
                                      in_=og[:, 0, :, :])
                    nc.sync.dma_start(out=outt[CLO:C, gb:gb + 2 * GP, :],
                                      in_=og[0:CHI, 1, :, :])

            LAG = 2
            pending = []

            def pop_back(st):
                emit_back(st)

            for g in range(NGRP):
                gb = 2 * GP * g
                xgb = xin.tile([CLO, 2, 2 * GP, 2, N], BF16, tag="xgb")
                nc.sync.dma_start(out=xgb[:, 0, :, :, :],
                                  in_=xyb[0:CLO, gb:gb + 2 * GP, :, :])
                nc.sync.dma_start(out=xgb[0:CHIA, 1, :, :, :],
                                  in_=xyb[CLO:C + 1, gb:gb + 2 * GP, :, :])
                og = outp.tile([CLO, 2, 2 * GP, N], BF16, tag="og")
                for u in range(GP):
                    pending.append(emit_front(GP * g + u, xgb, og))
                    if len(pending) > LAG:
                        pop_back(pending.pop(0))
            for st in pending:
                pop_back(st)

    nc.compile()
    return nc


def _host_prepack(d):
    """Fold BN, collapse the linear tail, build constant blobs."""
    f = np.float32
    bf = ml_dtypes.bfloat16
    inv1 = d["g1"] / np.sqrt(d["v1"] + 1e-5)
    W1 = (d["w1"] * inv1[:, None]).astype(f)
    c1 = ((d["b1"] - d["m1"]) * inv1 + d["be1"]).astype(f)
    inv2 = d["g2"] / np.sqrt(d["v2"] + 1e-5)
    W2 = (d["w2"] * inv2[:, None]).astype(f)
    c2 = ((d["b2"] - d["m2"]) * inv2 + d["be2"]).astype(f)

    w4eff = d["w4"][:, :L] + d["w4"][:, L:]
    w5a, w5b = d["w5"][0, :C], d["w5"][0, C:]
    u3 = (w5a @ d["w3"]).astype(f)
    u4 = (w5b @ w4eff).astype(f)
    K = float(w5a @ d["b3"] + w5b @ d["b4"] + d["b5"][0])

    W1T = np.ascontiguousarray(W1.T)
    W2T = np.ascontiguousarray(W2.T)

    wblob = np.zeros((CLO, 772), bf)

    def pack_stream(c0, WT, cb):
        # A [128,128]
        wblob[:, c0:c0 + 128] = WT[0:128, 0:128].astype(bf)
        # B [128,65] (out-hi; ones-unit comes via row64 of D)
        wblob[:, c0 + 128:c0 + 192] = WT[0:128, 128:192].astype(bf)
        # C [65,128]: rows 0:64 = WT hi-contraction, row 64 = bias
        wblob[0:CHI, c0 + 193:c0 + 321] = WT[128:192, 0:128].astype(bf)
        wblob[CHI, c0 + 193:c0 + 321] = cb[0:128].astype(bf)
        # D [65,65]
        wblob[0:CHI, c0 + 321:c0 + 385] = WT[128:192, 128:192].astype(bf)
        wblob[CHI, c0 + 321:c0 + 385] = cb[128:192].astype(bf)
        wblob[CHI, c0 + 385] = bf(1.0)

    pack_stream(0, W1T, c1)
    pack_stream(386, W2T, c2)

    u4b = np.broadcast_to(u4.astype(bf), (CLO, L)).copy()
    u34 = np.zeros((CLO, 4), f)
    u34[:, 0] = u34[:, 1] = u3[0:128]
    u34[0:CHI, 2] = u34[0:CHI, 3] = u3[128:192]
    u34[CHI, 2] = u34[CHI, 3] = K - float(u4.sum())
    return {"wblob": wblob, "u4b": u4b, "u34": u34}


def run(inputs, trace=False):
    d = {k: np.asarray(v) for k, v in inputs.items()}
    consts = _host_prepack(d)
    bf = ml_dtypes.bfloat16

    xyb = np.ones((C + 1, B, 2, N), bf)
    xyb[0:C, :, 0, :] = d["x"].transpose(2, 0, 1).astype(bf)
    xyb[0:C, :, 1, :] = d["y"].transpose(2, 0, 1).astype(bf)

    if "nc" not in _CACHE:
        _CACHE["nc"] = _build_program()
    nc = _CACHE["nc"]

    in_maps = []
    for cid in range(NCORES):
        m = dict(consts)
        m["xyb"] = np.ascontiguousarray(xyb[:, cid * NB:(cid + 1) * NB])
        in_maps.append(m)

    res = run_bass_kernel_spmd(nc, in_maps, list(range(NCORES)), trace=trace)
    # out HBM is [C, NB, N] bf16 channel-major; reassemble [B, C, N] f32
    out = np.concatenate(
        [res.results[i]["out"].astype(np.float32) for i in range(NCORES)], axis=1
    ).transpose(1, 0, 2)
    return np.ascontiguousarray(out), res


def kernel(**inputs):
    out, _ = run(inputs, trace=False)
    return out
